# revision 32
# baseline (speedup 1.0000x reference)
"""Trainium2 Bass kernel for nn_MultiHeadAttention_6081673691156.

Reference computation (N=4, SEQ=2048, EMBED=1024, H=16, D=64):
    k = keys.reshape(N, H, SEQ, D) @ Wk.T          (reshape, NOT transpose:
    v = values.reshape(...) @ Wv.T                  head h = contiguous memory
    q = queries.reshape(...) @ Wq.T                 block = rows 128h..128h+128
    e = (q @ k.T) / sqrt(EMBED)                     of the [SEQ, EMBED] matrix)
    e = where(mask==0, -1e20, e); a = softmax(e, -1)
    out = (a @ v).reshape(N, SEQ, EMBED) @ Wo.T + bo

Sharding: 8 cores = (batch n in 0..3) x (head half in 0..1); each core owns 8
heads of one batch.  NOTE the second reshape is also a flat memory
reinterpretation: output row q draws all 1024 features from head h=q//128 at
the 16 consecutive positions q'=16*(q%128)+t, so each core produces COMPLETE
output rows for its heads' row range — the host just concatenates row blocks
and adds bo.  The tiny DxD projections are folded into host-side input prep
(0.6% of FLOPs); masked softmax-attention and the output projection (99.4% of
FLOPs) run on device.

Per-core device pipeline (fp16 compute, f32 PSUM accumulation), blocked by
head-pairs x q-halves so the 2048x2048 score matrix never materializes:
  - S.T tile [128 l, 1024 q] = khatT x qhatT on PE (K=64)
  - exp on ScalarE, PSUM -> SBUF fp16   [bottleneck engine: 268M exps / 8]
  - mask multiply on DVE 2x mode (maskT streamed from HBM once per head-pair)
  - O = wT-chunks x [vhat | ones] on PE; ones column yields Z in psum col 64
  - normalize on DVE (per-partition 1/Z), transpose O-tiles on PE
  - partial out = attT x WoT-slice on PE, DVE evac, DMA -> DRAM
"""

import sys
from contextlib import ExitStack

import numpy as np

sys.path.insert(0, "/opt/trn_rl_repo")

import concourse.bass as bass  # noqa: E402
import concourse.tile as tile  # noqa: E402
from concourse import bacc, mybir  # noqa: E402

N_BATCH = 4
SEQ = 2048
EMBED = 1024
H = 16
D = 64
HPC = 8          # heads per core
N_CORES = 8
PAIRS = 4        # head pairs per core
LCH = 16         # l chunks of 128

FP16 = mybir.dt.float16
F32 = mybir.dt.float32


def build_program():
    nc = bacc.Bacc("TRN2", target_bir_lowering=False, debug=False)

    qT_d = nc.dram_tensor("qhatT", [HPC, D, SEQ], FP16, kind="ExternalInput").ap()
    kT_d = nc.dram_tensor("khatT", [HPC, D, SEQ], FP16, kind="ExternalInput").ap()
    vh_d = nc.dram_tensor("vhat", [HPC, 128, 16 * 65], FP16, kind="ExternalInput").ap()
    mT_d = nc.dram_tensor("maskT", [SEQ, SEQ], FP16, kind="ExternalInput").ap()
    woT_d = nc.dram_tensor("woT", [16, D, EMBED], FP16, kind="ExternalInput").ap()
    id_d = nc.dram_tensor("ident", [128, 128], FP16, kind="ExternalInput").ap()
    out_d = nc.dram_tensor("out", [HPC * 128, EMBED], F32, kind="ExternalOutput").ap()

    with tile.TileContext(nc) as tc:
        with ExitStack() as ctx:
            kern(ctx, tc, qT_d, kT_d, vh_d, mT_d, woT_d, id_d, out_d)
    nc.compile()
    return nc


def kern(ctx, tc, qT_d, kT_d, vh_d, mT_d, woT_d, id_d, out_d):
    nc = tc.nc
    Exp = mybir.ActivationFunctionType.Exp
    mult = mybir.AluOpType.mult

    # SBUF pools
    const_p = ctx.enter_context(tc.tile_pool(name="const", bufs=1))
    hat_p = ctx.enter_context(tc.tile_pool(name="hat", bufs=6))
    vhat_p = ctx.enter_context(tc.tile_pool(name="vhat", bufs=4))
    mask_p = ctx.enter_context(tc.tile_pool(name="mask", bufs=4))
    wt_p = ctx.enter_context(tc.tile_pool(name="wt", bufs=42))
    attT_p = ctx.enter_context(tc.tile_pool(name="attT", bufs=4))
    obar_p = ctx.enter_context(tc.tile_pool(name="obar", bufs=4))
    rz_p = ctx.enter_context(tc.tile_pool(name="rz", bufs=6))
    oev_p = ctx.enter_context(tc.tile_pool(name="oev", bufs=2))
    # PSUM pools: 4 + 2 + 2 = 8 banks
    psS_p = ctx.enter_context(tc.tile_pool(name="psS", bufs=2, space="PSUM"))
    psO_p = ctx.enter_context(tc.tile_pool(name="psO", bufs=2, space="PSUM"))
    psT_p = ctx.enter_context(tc.tile_pool(name="psT", bufs=2, space="PSUM"))

    # constants / weights: WoT row-blocks [64, 1024] for t = 0..15.
    # Loaded lazily (first use is after pair 0's attention) so the critical
    # first S matmuls aren't queued behind 17 constant DMAs.
    ident = const_p.tile([128, 128], FP16, tag="ident")
    woT = [const_p.tile([D, EMBED], FP16, tag=f"woT{t}", name=f"woT_{t}")
           for t in range(16)]

    # per-pair state, filled by load_pair / emit_S
    pair_state = {}

    def load_pair(p):
        h0 = 2 * p
        qhat, khat, vhat = [], [], []
        for hi in range(2):
            h = h0 + hi
            qh_t = hat_p.tile([D, SEQ], FP16, tag="qhat", name=f"qhat_{h}")
            nc.sync.dma_start(qh_t[:, :], qT_d[h, :, :])
            kh_t = hat_p.tile([D, SEQ], FP16, tag="khat", name=f"khat_{h}")
            nc.sync.dma_start(kh_t[:, :], kT_d[h, :, :])
            qhat.append(qh_t)
            khat.append(kh_t)
            vt = vhat_p.tile([128, 16 * 65], FP16, tag="vhat", name=f"vhat_{h}")
            nc.sync.dma_start(vt[:, :], vh_d[h, :, :])
            vhat.append(vt)
        aT = [attT_p.tile([D, SEQ], FP16, tag="attT", name=f"attT_{p}_{i}")
              for i in range(2)]
        pair_state[p] = dict(qhat=qhat, khat=khat, vhat=vhat, aT=aT)

    def emit_S_unit(stage, l, wts):
        """score + exp + mask for one l chunk (both heads) of (pair, qh)."""
        p, qh = stage
        st = pair_state[p]
        mt = mask_p.tile([128, 1024], FP16, tag="mask", name=f"m_{p}_{qh}_{l}")
        nc.sync.dma_start(mt[:, :],
                          mT_d[128 * l:128 * (l + 1),
                               1024 * qh:1024 * (qh + 1)])
        for hi in range(2):
            psS = psS_p.tile([128, 1024], F32, tag="ps_s",
                             name=f"psS_{p}_{qh}_{l}_{hi}")
            lk = st["khat"][hi][:, 128 * l:128 * (l + 1)]
            for c in range(2):
                nc.tensor.matmul(
                    psS[:, 512 * c:512 * (c + 1)], lhsT=lk,
                    rhs=st["qhat"][hi][:, 1024 * qh + 512 * c:
                                       1024 * qh + 512 * (c + 1)],
                    start=True, stop=True)
            wt = wt_p.tile([128, 1024], FP16, tag="wt",
                           name=f"wt_{p}_{qh}_{l}_{hi}")
            nc.scalar.activation(wt[:, :], psS[:, :], Exp)
            nc.vector.tensor_tensor(out=wt[:, :], in0=wt[:, :],
                                    in1=mt[:, :], op=mult)
            wts[hi][l] = wt

    def emit_O(stage, hi, wts, unit_iter, nxt, nxt_wts):
        """attention-weighted V + normalize + transpose for one head.
        After each 16-matmul accumulation group, one next-stage S unit is
        emitted so the in-order PE stream always has exp producers queued
        (keeps ScalarE, the bottleneck engine, saturated)."""
        p, qh = stage
        st = pair_state[p]
        for g in range(2):
            psT = psT_p.tile([D, 512], FP16, tag="ps_t",
                             name=f"psT_{p}_{qh}_{hi}_{g}")
            for k in range(4):
                qt = 4 * g + k
                psO = psO_p.tile([128, 65], F32, tag="ps_o",
                                 name=f"psO_{p}_{qh}_{hi}_{qt}")
                for l in range(LCH):
                    nc.tensor.matmul(
                        psO[:, :],
                        lhsT=wts[hi][l][:, 128 * qt:128 * (qt + 1)],
                        rhs=st["vhat"][hi][:, 65 * l:65 * (l + 1)],
                        start=(l == 0), stop=(l == LCH - 1))
                rz = rz_p.tile([128, 1], F32, tag="rz",
                               name=f"rz_{p}_{qh}_{hi}_{qt}")
                nc.vector.reciprocal(rz[:, :], psO[:, 64:65])
                ob = obar_p.tile([128, D], FP16, tag="obar",
                                 name=f"ob_{p}_{qh}_{hi}_{qt}")
                nc.vector.tensor_scalar_mul(ob[:, :], psO[:, 0:D], rz[:, 0:1])
                nc.tensor.transpose(psT[:, 128 * k:128 * (k + 1)],
                                    ob[:, :], ident[:, :])
                l_nxt = next(unit_iter, None)
                if l_nxt is not None:
                    emit_S_unit(nxt, l_nxt, nxt_wts)
            nc.vector.tensor_copy(
                st["aT"][hi][:, 1024 * qh + 512 * g:1024 * qh + 512 * (g + 1)],
                psT[:, :])

    def emit_Wo(p, hi, unit_iter=iter(()), nxt=None, nxt_wts=None):
        """output projection for head 2p+hi (needs aT[hi] complete).
        out row 128h+b uses head h features A_h[16b+t, d] -> Wo.T[64t+d]:
        out[128h.., e] = sum_t A_h.T[:, t::16].T @ WoT[64t:64t+64, :]"""
        h = 2 * p + hi
        aTr = pair_state[p]["aT"][hi][:, :].rearrange("d (b t) -> d t b", t=16)
        for e in range(2):
            es = slice(512 * e, 512 * (e + 1))
            psW = psO_p.tile([128, 512], F32, tag="ps_o", name=f"psW_{h}_{e}")
            for t in range(16):
                nc.tensor.matmul(psW[:, :], lhsT=aTr[:, t, :],
                                 rhs=woT[t][:, es],
                                 start=(t == 0), stop=(t == 15))
            ov = oev_p.tile([128, 512], F32, tag="oev", name=f"ov_{h}_{e}")
            nc.vector.tensor_copy(ov[:, :], psW[:, :])
            nc.sync.dma_start(out_d[128 * h:128 * (h + 1), es], ov[:, :])
            l_nxt = next(unit_iter, None)
            if l_nxt is not None:
                emit_S_unit(nxt, l_nxt, nxt_wts)

    # Software pipeline over 8 stages (pair, q-half): the next stage's
    # S/exp/mask work is emitted between the current stage's two per-head
    # O-phases so the in-order PE stream always has exp producers queued
    # while O-accumulation runs (keeps ScalarE, the bottleneck, saturated).
    stages = [(p, qh) for p in range(PAIRS) for qh in range(2)]
    load_pair(0)
    cur = [[None] * LCH, [None] * LCH]
    for l in range(LCH):
        emit_S_unit(stages[0], l, cur)
    nc.sync.dma_start(ident[:, :], id_d[:, :])
    for t in range(16):
        nc.sync.dma_start(woT[t][:, :], woT_d[t, :, :])

    for idx, stage in enumerate(stages):
        p, qh = stage
        nxt = stages[idx + 1] if idx + 1 < len(stages) else None
        nxt_wts = [[None] * LCH, [None] * LCH] if nxt else None
        if nxt and nxt[1] == 0:
            load_pair(nxt[0])
        unit_iter = iter(range(LCH)) if nxt else iter(())
        emit_O(stage, 0, cur, unit_iter, nxt, nxt_wts)
        if qh == 1:
            emit_Wo(p, 0)
        emit_O(stage, 1, cur, unit_iter, nxt, nxt_wts)
        if qh == 1:
            emit_Wo(p, 1)
        cur = nxt_wts


_NC_CACHE = None


def get_nc():
    global _NC_CACHE
    if _NC_CACHE is None:
        _NC_CACHE = build_program()
    return _NC_CACHE


def make_in_maps(keys, values, queries, mask, Wk, Wv, Wq, Wo, bo):
    keys = np.asarray(keys, np.float32)
    values = np.asarray(values, np.float32)
    queries = np.asarray(queries, np.float32)
    mask = np.asarray(mask)
    Wk = np.asarray(Wk, np.float32)
    Wv = np.asarray(Wv, np.float32)
    Wq = np.asarray(Wq, np.float32)
    Wo = np.asarray(Wo, np.float32)

    ident = np.eye(128, dtype=np.float16)
    woT = np.ascontiguousarray(Wo.T.astype(np.float16)).reshape(16, D, EMBED)
    wq_s = (Wq / 32.0).astype(np.float32)           # fold 1/sqrt(EMBED) into q

    in_maps = []
    for n in range(N_BATCH):
        maskT = np.ascontiguousarray(mask[n, 0].T).astype(np.float16)
        for half in range(2):
            rows = slice(half * 1024, (half + 1) * 1024)
            # heads of this core as [8, 2048, 64] blocks
            qb = queries[n, rows, :].reshape(HPC, SEQ, D)
            kb = keys[n, rows, :].reshape(HPC, SEQ, D)
            vb = values[n, rows, :].reshape(HPC, SEQ, D)
            # host projections: qhatT/khatT as [8, 64(dout), 2048(l)]
            qhatT = np.einsum("od,hld->hol", wq_s, qb).astype(np.float16)
            khatT = np.einsum("od,hld->hol", Wk, kb).astype(np.float16)
            vhat = vb @ Wv.T                        # [8, 2048, 64] f32
            vext = np.empty((HPC, SEQ, 65), np.float16)
            vext[:, :, :D] = vhat.astype(np.float16)
            vext[:, :, D] = 1.0
            # device layout [8, 128, 16*65]: row p, block j -> l = 128*j + p
            vsh = np.ascontiguousarray(
                vext.reshape(HPC, 16, 128, 65).transpose(0, 2, 1, 3)
            ).reshape(HPC, 128, 16 * 65)
            in_maps.append({
                "qhatT": np.ascontiguousarray(qhatT),
                "khatT": np.ascontiguousarray(khatT),
                "vhat": vsh, "maskT": maskT,
                "woT": woT, "ident": ident,
            })
    return in_maps


def kernel(keys, values, queries, mask, Wk, Wv, Wq, Wo, bo):
    from concourse.bass_utils import run_bass_kernel_spmd

    nc = get_nc()
    in_maps = make_in_maps(keys, values, queries, mask, Wk, Wv, Wq, Wo, bo)
    res = run_bass_kernel_spmd(nc, in_maps, core_ids=list(range(N_CORES)))
    parts = [r["out"] for r in res.results]
    bo = np.asarray(bo, np.float32)
    out = np.empty((N_BATCH, SEQ, EMBED), np.float32)
    for n in range(N_BATCH):
        out[n, :1024] = parts[2 * n] + bo
        out[n, 1024:] = parts[2 * n + 1] + bo
    return out


# revision 42
# speedup vs baseline: 1.0164x; 1.0164x over previous
"""Trainium2 Bass kernel for nn_MultiHeadAttention_6081673691156.

Reference computation (N=4, SEQ=2048, EMBED=1024, H=16, D=64):
    k = keys.reshape(N, H, SEQ, D) @ Wk.T          (reshape, NOT transpose:
    v = values.reshape(...) @ Wv.T                  head h = contiguous memory
    q = queries.reshape(...) @ Wq.T                 block = rows 128h..128h+128
    e = (q @ k.T) / sqrt(EMBED)                     of the [SEQ, EMBED] matrix)
    e = where(mask==0, -1e20, e); a = softmax(e, -1)
    out = (a @ v).reshape(N, SEQ, EMBED) @ Wo.T + bo

Sharding: 8 cores = (batch n in 0..3) x (head half in 0..1); each core owns 8
heads of one batch.  NOTE the second reshape is also a flat memory
reinterpretation: output row q draws all 1024 features from head h=q//128 at
the 16 consecutive positions q'=16*(q%128)+t, so each core produces COMPLETE
output rows for its heads' row range — the host just concatenates row blocks
and adds bo.  The tiny DxD projections are folded into host-side input prep
(0.6% of FLOPs); masked softmax-attention and the output projection (99.4% of
FLOPs) run on device.

Per-core device pipeline (fp16 compute, f32 PSUM accumulation), blocked by
head-pairs x q-halves so the 2048x2048 score matrix never materializes:
  - S.T tile [128 l, 1024 q] = khatT x qhatT on PE (K=64)
  - exp on ScalarE, PSUM -> SBUF fp16   [bottleneck engine: 268M exps / 8]
  - mask multiply on DVE 2x mode (maskT streamed from HBM once per head-pair)
  - O = wT-chunks x [vhat | ones] on PE; ones column yields Z in psum col 64
  - normalize on DVE (per-partition 1/Z), transpose O-tiles on PE
  - partial out = attT x WoT-slice on PE, DVE evac, DMA -> DRAM
"""

import sys
from contextlib import ExitStack

import numpy as np

sys.path.insert(0, "/opt/trn_rl_repo")

import concourse.bass as bass  # noqa: E402
import concourse.tile as tile  # noqa: E402
from concourse import bacc, mybir  # noqa: E402

N_BATCH = 4
SEQ = 2048
EMBED = 1024
H = 16
D = 64
HPC = 8          # heads per core
N_CORES = 8
PAIRS = 4        # head pairs per core
LCH = 16         # l chunks of 128

FP16 = mybir.dt.float16
F32 = mybir.dt.float32


def build_program():
    nc = bacc.Bacc("TRN2", target_bir_lowering=False, debug=False)

    qT_d = nc.dram_tensor("qhatT", [HPC, D, SEQ], FP16, kind="ExternalInput").ap()
    kT_d = nc.dram_tensor("khatT", [HPC, D, SEQ], FP16, kind="ExternalInput").ap()
    vh_d = nc.dram_tensor("vhat", [HPC, 128, 16 * 65], FP16, kind="ExternalInput").ap()
    mT_d = nc.dram_tensor("maskT", [SEQ, SEQ], FP16, kind="ExternalInput").ap()
    woT_d = nc.dram_tensor("woT", [16, D, EMBED], FP16, kind="ExternalInput").ap()
    id_d = nc.dram_tensor("ident", [128, 128], FP16, kind="ExternalInput").ap()
    out_d = nc.dram_tensor("out", [HPC * 128, EMBED], F32, kind="ExternalOutput").ap()

    with tile.TileContext(nc) as tc:
        with ExitStack() as ctx:
            kern(ctx, tc, qT_d, kT_d, vh_d, mT_d, woT_d, id_d, out_d)
    nc.compile()
    return nc


def kern(ctx, tc, qT_d, kT_d, vh_d, mT_d, woT_d, id_d, out_d):
    nc = tc.nc
    Exp = mybir.ActivationFunctionType.Exp
    mult = mybir.AluOpType.mult

    # SBUF pools
    const_p = ctx.enter_context(tc.tile_pool(name="const", bufs=1))
    hat_p = ctx.enter_context(tc.tile_pool(name="hat", bufs=6))
    vhat_p = ctx.enter_context(tc.tile_pool(name="vhat", bufs=4))
    mask_p = ctx.enter_context(tc.tile_pool(name="mask", bufs=4))
    wt_p = ctx.enter_context(tc.tile_pool(name="wt", bufs=42))
    attT_p = ctx.enter_context(tc.tile_pool(name="attT", bufs=4))
    obar_p = ctx.enter_context(tc.tile_pool(name="obar", bufs=4))
    rz_p = ctx.enter_context(tc.tile_pool(name="rz", bufs=6))
    oev_p = ctx.enter_context(tc.tile_pool(name="oev", bufs=2))
    # PSUM pools: 4 + 2 + 2 = 8 banks
    psS_p = ctx.enter_context(tc.tile_pool(name="psS", bufs=2, space="PSUM"))
    psO_p = ctx.enter_context(tc.tile_pool(name="psO", bufs=3, space="PSUM"))
    psT_p = ctx.enter_context(tc.tile_pool(name="psT", bufs=1, space="PSUM"))

    # constants / weights: WoT row-blocks [64, 1024] for t = 0..15.
    # Loaded lazily (first use is after pair 0's attention) so the critical
    # first S matmuls aren't queued behind 17 constant DMAs.
    ident = const_p.tile([128, 128], FP16, tag="ident")
    woT = [const_p.tile([D, EMBED], FP16, tag=f"woT{t}", name=f"woT_{t}")
           for t in range(16)]

    # per-pair state, filled by load_pair / emit_S
    pair_state = {}

    def load_pair(p):
        h0 = 2 * p
        qhat, khat, vhat = [], [], []
        for hi in range(2):
            h = h0 + hi
            qh_t = hat_p.tile([D, SEQ], FP16, tag="qhat", name=f"qhat_{h}")
            nc.sync.dma_start(qh_t[:, :], qT_d[h, :, :])
            kh_t = hat_p.tile([D, SEQ], FP16, tag="khat", name=f"khat_{h}")
            nc.sync.dma_start(kh_t[:, :], kT_d[h, :, :])
            qhat.append(qh_t)
            khat.append(kh_t)
            vt = vhat_p.tile([128, 16 * 65], FP16, tag="vhat", name=f"vhat_{h}")
            nc.sync.dma_start(vt[:, :], vh_d[h, :, :])
            vhat.append(vt)
        aT = [attT_p.tile([D, SEQ], FP16, tag="attT", name=f"attT_{p}_{i}")
              for i in range(2)]
        pair_state[p] = dict(qhat=qhat, khat=khat, vhat=vhat, aT=aT)

    def emit_S_unit(stage, l, wts):
        """score + exp + mask for one l chunk (both heads) of (pair, qh)."""
        p, qh = stage
        st = pair_state[p]
        mt = mask_p.tile([128, 1024], FP16, tag="mask", name=f"m_{p}_{qh}_{l}")
        nc.sync.dma_start(mt[:, :],
                          mT_d[128 * l:128 * (l + 1),
                               1024 * qh:1024 * (qh + 1)])
        for hi in range(2):
            psS = psS_p.tile([128, 1024], F32, tag="ps_s",
                             name=f"psS_{p}_{qh}_{l}_{hi}")
            lk = st["khat"][hi][:, 128 * l:128 * (l + 1)]
            for c in range(2):
                nc.tensor.matmul(
                    psS[:, 512 * c:512 * (c + 1)], lhsT=lk,
                    rhs=st["qhat"][hi][:, 1024 * qh + 512 * c:
                                       1024 * qh + 512 * (c + 1)],
                    start=True, stop=True)
            wt = wt_p.tile([128, 1024], FP16, tag="wt",
                           name=f"wt_{p}_{qh}_{l}_{hi}")
            nc.scalar.activation(wt[:, :], psS[:, :], Exp)
            nc.vector.tensor_tensor(out=wt[:, :], in0=wt[:, :],
                                    in1=mt[:, :], op=mult)
            wts[hi][l] = wt

    def emit_O(stage, hi, wts, unit_iter, nxt, nxt_wts):
        """attention-weighted V + normalize + transpose for one head.
        After each 16-matmul accumulation group, one next-stage S unit is
        emitted so the in-order PE stream always has exp producers queued
        (keeps ScalarE, the bottleneck engine, saturated)."""
        p, qh = stage
        st = pair_state[p]
        for g in range(2):
            psT = psT_p.tile([D, 512], FP16, tag="ps_t",
                             name=f"psT_{p}_{qh}_{hi}_{g}")
            for k in range(4):
                qt = 4 * g + k
                psO = psO_p.tile([128, 65], F32, tag="ps_o",
                                 name=f"psO_{p}_{qh}_{hi}_{qt}")
                for l in range(LCH):
                    nc.tensor.matmul(
                        psO[:, :],
                        lhsT=wts[hi][l][:, 128 * qt:128 * (qt + 1)],
                        rhs=st["vhat"][hi][:, 65 * l:65 * (l + 1)],
                        start=(l == 0), stop=(l == LCH - 1))
                rz = rz_p.tile([128, 1], F32, tag="rz",
                               name=f"rz_{p}_{qh}_{hi}_{qt}")
                nc.vector.reciprocal(rz[:, :], psO[:, 64:65])
                ob = obar_p.tile([128, D], FP16, tag="obar",
                                 name=f"ob_{p}_{qh}_{hi}_{qt}")
                nc.vector.tensor_scalar_mul(ob[:, :], psO[:, 0:D], rz[:, 0:1])
                nc.tensor.transpose(psT[:, 128 * k:128 * (k + 1)],
                                    ob[:, :], ident[:, :])
                l_nxt = next(unit_iter, None)
                if l_nxt is not None:
                    emit_S_unit(nxt, l_nxt, nxt_wts)
            nc.vector.tensor_copy(
                st["aT"][hi][:, 1024 * qh + 512 * g:1024 * qh + 512 * (g + 1)],
                psT[:, :])

    def emit_Wo(p, hi, unit_iter=iter(()), nxt=None, nxt_wts=None):
        """output projection for head 2p+hi (needs aT[hi] complete).
        out row 128h+b uses head h features A_h[16b+t, d] -> Wo.T[64t+d]:
        out[128h.., e] = sum_t A_h.T[:, t::16].T @ WoT[64t:64t+64, :]"""
        h = 2 * p + hi
        aTr = pair_state[p]["aT"][hi][:, :].rearrange("d (b t) -> d t b", t=16)
        for e in range(2):
            es = slice(512 * e, 512 * (e + 1))
            psW = psO_p.tile([128, 512], F32, tag="ps_o", name=f"psW_{h}_{e}")
            for t in range(16):
                nc.tensor.matmul(psW[:, :], lhsT=aTr[:, t, :],
                                 rhs=woT[t][:, es],
                                 start=(t == 0), stop=(t == 15))
            ov = oev_p.tile([128, 512], F32, tag="oev", name=f"ov_{h}_{e}")
            nc.vector.tensor_copy(ov[:, :], psW[:, :])
            nc.sync.dma_start(out_d[128 * h:128 * (h + 1), es], ov[:, :])
            l_nxt = next(unit_iter, None)
            if l_nxt is not None:
                emit_S_unit(nxt, l_nxt, nxt_wts)

    # Software pipeline over 8 stages (pair, q-half): the next stage's
    # S/exp/mask work is emitted between the current stage's two per-head
    # O-phases so the in-order PE stream always has exp producers queued
    # while O-accumulation runs (keeps ScalarE, the bottleneck, saturated).
    stages = [(p, qh) for p in range(PAIRS) for qh in range(2)]
    # warm up ScalarE first: the one-time exp table-set load (~2.7us) runs
    # against a memset scratch tile while the first input DMAs are in flight
    warm = obar_p.tile([128, 1], FP16, tag="obar", name="act_warm")
    nc.gpsimd.memset(warm[:, :], 0.0)
    nc.scalar.activation(warm[:, :], warm[:, :], Exp)
    load_pair(0)
    cur = [[None] * LCH, [None] * LCH]
    for l in range(LCH):
        emit_S_unit(stages[0], l, cur)
    nc.sync.dma_start(ident[:, :], id_d[:, :])
    for t in range(16):
        nc.sync.dma_start(woT[t][:, :], woT_d[t, :, :])

    for idx, stage in enumerate(stages):
        p, qh = stage
        nxt = stages[idx + 1] if idx + 1 < len(stages) else None
        nxt_wts = [[None] * LCH, [None] * LCH] if nxt else None
        if nxt and nxt[1] == 0:
            load_pair(nxt[0])
        unit_iter = iter(range(LCH)) if nxt else iter(())
        emit_O(stage, 0, cur, unit_iter, nxt, nxt_wts)
        if qh == 1:
            emit_Wo(p, 0)
        emit_O(stage, 1, cur, unit_iter, nxt, nxt_wts)
        if qh == 1:
            emit_Wo(p, 1)
        cur = nxt_wts


_NC_CACHE = None


def get_nc():
    global _NC_CACHE
    if _NC_CACHE is None:
        _NC_CACHE = build_program()
    return _NC_CACHE


def make_in_maps(keys, values, queries, mask, Wk, Wv, Wq, Wo, bo):
    keys = np.asarray(keys, np.float32)
    values = np.asarray(values, np.float32)
    queries = np.asarray(queries, np.float32)
    mask = np.asarray(mask)
    Wk = np.asarray(Wk, np.float32)
    Wv = np.asarray(Wv, np.float32)
    Wq = np.asarray(Wq, np.float32)
    Wo = np.asarray(Wo, np.float32)

    ident = np.eye(128, dtype=np.float16)
    woT = np.ascontiguousarray(Wo.T.astype(np.float16)).reshape(16, D, EMBED)
    wq_s = (Wq / 32.0).astype(np.float32)           # fold 1/sqrt(EMBED) into q

    in_maps = []
    for n in range(N_BATCH):
        maskT = np.ascontiguousarray(mask[n, 0].T).astype(np.float16)
        for half in range(2):
            rows = slice(half * 1024, (half + 1) * 1024)
            # heads of this core as [8, 2048, 64] blocks
            qb = queries[n, rows, :].reshape(HPC, SEQ, D)
            kb = keys[n, rows, :].reshape(HPC, SEQ, D)
            vb = values[n, rows, :].reshape(HPC, SEQ, D)
            # host projections: qhatT/khatT as [8, 64(dout), 2048(l)]
            qhatT = np.einsum("od,hld->hol", wq_s, qb).astype(np.float16)
            khatT = np.einsum("od,hld->hol", Wk, kb).astype(np.float16)
            vhat = vb @ Wv.T                        # [8, 2048, 64] f32
            vext = np.empty((HPC, SEQ, 65), np.float16)
            vext[:, :, :D] = vhat.astype(np.float16)
            vext[:, :, D] = 1.0
            # device layout [8, 128, 16*65]: row p, block j -> l = 128*j + p
            vsh = np.ascontiguousarray(
                vext.reshape(HPC, 16, 128, 65).transpose(0, 2, 1, 3)
            ).reshape(HPC, 128, 16 * 65)
            in_maps.append({
                "qhatT": np.ascontiguousarray(qhatT),
                "khatT": np.ascontiguousarray(khatT),
                "vhat": vsh, "maskT": maskT,
                "woT": woT, "ident": ident,
            })
    return in_maps


def kernel(keys, values, queries, mask, Wk, Wv, Wq, Wo, bo):
    from concourse.bass_utils import run_bass_kernel_spmd

    nc = get_nc()
    in_maps = make_in_maps(keys, values, queries, mask, Wk, Wv, Wq, Wo, bo)
    res = run_bass_kernel_spmd(nc, in_maps, core_ids=list(range(N_CORES)))
    parts = [r["out"] for r in res.results]
    bo = np.asarray(bo, np.float32)
    out = np.empty((N_BATCH, SEQ, EMBED), np.float32)
    for n in range(N_BATCH):
        out[n, :1024] = parts[2 * n] + bo
        out[n, 1024:] = parts[2 * n + 1] + bo
    return out


# revision 45
# speedup vs baseline: 1.0511x; 1.0341x over previous
"""Trainium2 Bass kernel for nn_MultiHeadAttention_6081673691156.

Reference computation (N=4, SEQ=2048, EMBED=1024, H=16, D=64):
    k = keys.reshape(N, H, SEQ, D) @ Wk.T          (reshape, NOT transpose:
    v = values.reshape(...) @ Wv.T                  head h = contiguous memory
    q = queries.reshape(...) @ Wq.T                 block = rows 128h..128h+128
    e = (q @ k.T) / sqrt(EMBED)                     of the [SEQ, EMBED] matrix)
    e = where(mask==0, -1e20, e); a = softmax(e, -1)
    out = (a @ v).reshape(N, SEQ, EMBED) @ Wo.T + bo

Sharding: 8 cores = (batch n in 0..3) x (head half in 0..1); each core owns 8
heads of one batch.  NOTE the second reshape is also a flat memory
reinterpretation: output row q draws all 1024 features from head h=q//128 at
the 16 consecutive positions q'=16*(q%128)+t, so each core produces COMPLETE
output rows for its heads' row range — the host just concatenates row blocks
and adds bo.  The tiny DxD projections are folded into host-side input prep
(0.6% of FLOPs); masked softmax-attention and the output projection (99.4% of
FLOPs) run on device.

Per-core device pipeline (fp16 compute, f32 PSUM accumulation), blocked by
head-pairs x q-halves so the 2048x2048 score matrix never materializes:
  - S.T tile [128 l, 1024 q] = khatT x qhatT on PE (K=64)
  - exp on ScalarE, PSUM -> SBUF fp16   [bottleneck engine: 268M exps / 8]
  - mask multiply on DVE 2x mode (maskT streamed from HBM once per head-pair)
  - O = wT-chunks x [vhat | ones] on PE; ones column yields Z in psum col 64
  - normalize on DVE (per-partition 1/Z), transpose O-tiles on PE
  - partial out = attT x WoT-slice on PE, DVE evac, DMA -> DRAM
"""

import sys
from contextlib import ExitStack

import numpy as np

sys.path.insert(0, "/opt/trn_rl_repo")

import concourse.bass as bass  # noqa: E402
import concourse.tile as tile  # noqa: E402
from concourse import bacc, mybir  # noqa: E402

N_BATCH = 4
SEQ = 2048
EMBED = 1024
H = 16
D = 64
HPC = 8          # heads per core
N_CORES = 8
PAIRS = 4        # head pairs per core
LCH = 16         # l chunks of 128

FP16 = mybir.dt.float16
F32 = mybir.dt.float32


def build_program():
    nc = bacc.Bacc("TRN2", target_bir_lowering=False, debug=False)

    qT_d = nc.dram_tensor("qhatT", [HPC, D, SEQ], FP16, kind="ExternalInput").ap()
    kT_d = nc.dram_tensor("khatT", [HPC, D, SEQ], FP16, kind="ExternalInput").ap()
    vh_d = nc.dram_tensor("vhat", [HPC, 128, 16 * 65], FP16, kind="ExternalInput").ap()
    mT_d = nc.dram_tensor("maskT", [SEQ, SEQ], FP16, kind="ExternalInput").ap()
    woT_d = nc.dram_tensor("woT", [16, D, EMBED], FP16, kind="ExternalInput").ap()
    id_d = nc.dram_tensor("ident", [128, 128], FP16, kind="ExternalInput").ap()
    out_d = nc.dram_tensor("out", [HPC * 128, EMBED], F32, kind="ExternalOutput").ap()

    with tile.TileContext(nc) as tc:
        with ExitStack() as ctx:
            kern(ctx, tc, qT_d, kT_d, vh_d, mT_d, woT_d, id_d, out_d)
    nc.compile()
    return nc


def kern(ctx, tc, qT_d, kT_d, vh_d, mT_d, woT_d, id_d, out_d):
    nc = tc.nc
    Exp = mybir.ActivationFunctionType.Exp
    mult = mybir.AluOpType.mult

    # SBUF pools
    const_p = ctx.enter_context(tc.tile_pool(name="const", bufs=1))
    hat_p = ctx.enter_context(tc.tile_pool(name="hat", bufs=6))
    vhat_p = ctx.enter_context(tc.tile_pool(name="vhat", bufs=4))
    mask_p = ctx.enter_context(tc.tile_pool(name="mask", bufs=4))
    wt_p = ctx.enter_context(tc.tile_pool(name="wt", bufs=42))
    attT_p = ctx.enter_context(tc.tile_pool(name="attT", bufs=4))
    obar_p = ctx.enter_context(tc.tile_pool(name="obar", bufs=4))
    rz_p = ctx.enter_context(tc.tile_pool(name="rz", bufs=6))
    oev_p = ctx.enter_context(tc.tile_pool(name="oev", bufs=2))
    # PSUM pools: 4 + 2 + 2 = 8 banks
    psS_p = ctx.enter_context(tc.tile_pool(name="psS", bufs=2, space="PSUM"))
    psO_p = ctx.enter_context(tc.tile_pool(name="psO", bufs=3, space="PSUM"))
    psT_p = ctx.enter_context(tc.tile_pool(name="psT", bufs=1, space="PSUM"))

    # constants / weights: WoT row-blocks [64, 1024] for t = 0..15.
    # Loaded lazily (first use is after pair 0's attention) so the critical
    # first S matmuls aren't queued behind 17 constant DMAs.
    ident = const_p.tile([128, 128], FP16, tag="ident")
    woT = [const_p.tile([D, EMBED], FP16, tag=f"woT{t}", name=f"woT_{t}")
           for t in range(16)]

    # per-pair state, filled by load_pair / emit_S
    pair_state = {}

    def load_pair(p):
        h0 = 2 * p
        qhat, khat, vhat = [], [], []
        for hi in range(2):
            h = h0 + hi
            qh_t = hat_p.tile([D, SEQ], FP16, tag="qhat", name=f"qhat_{h}")
            nc.sync.dma_start(qh_t[:, :], qT_d[h, :, :])
            kh_t = hat_p.tile([D, SEQ], FP16, tag="khat", name=f"khat_{h}")
            nc.sync.dma_start(kh_t[:, :], kT_d[h, :, :])
            qhat.append(qh_t)
            khat.append(kh_t)
            vt = vhat_p.tile([128, 16 * 65], FP16, tag="vhat", name=f"vhat_{h}")
            nc.sync.dma_start(vt[:, :], vh_d[h, :, :])
            vhat.append(vt)
        aT = [attT_p.tile([D, SEQ], FP16, tag="attT", name=f"attT_{p}_{i}")
              for i in range(2)]
        pair_state[p] = dict(qhat=qhat, khat=khat, vhat=vhat, aT=aT)

    def emit_S_unit(stage, l, wts):
        """score + exp + mask for one l chunk (both heads) of (pair, qh)."""
        p, qh = stage
        st = pair_state[p]
        mt = mask_p.tile([128, 1024], FP16, tag="mask", name=f"m_{p}_{qh}_{l}")
        nc.sync.dma_start(mt[:, :],
                          mT_d[128 * l:128 * (l + 1),
                               1024 * qh:1024 * (qh + 1)])
        for hi in range(2):
            psS = psS_p.tile([128, 1024], F32, tag="ps_s",
                             name=f"psS_{p}_{qh}_{l}_{hi}")
            lk = st["khat"][hi][:, 128 * l:128 * (l + 1)]
            for c in range(2):
                nc.tensor.matmul(
                    psS[:, 512 * c:512 * (c + 1)], lhsT=lk,
                    rhs=st["qhat"][hi][:, 1024 * qh + 512 * c:
                                       1024 * qh + 512 * (c + 1)],
                    start=True, stop=True)
            wt = wt_p.tile([128, 1024], FP16, tag="wt",
                           name=f"wt_{p}_{qh}_{l}_{hi}")
            nc.scalar.activation(wt[:, :], psS[:, :], Exp)
            nc.vector.tensor_tensor(out=wt[:, :], in0=wt[:, :],
                                    in1=mt[:, :], op=mult)
            wts[hi][l] = wt

    def emit_O(stage, hi, wts, unit_iter, nxt, nxt_wts):
        """attention-weighted V + normalize + transpose for one head.
        After each 16-matmul accumulation group, one next-stage S unit is
        emitted so the in-order PE stream always has exp producers queued
        (keeps ScalarE, the bottleneck engine, saturated)."""
        p, qh = stage
        st = pair_state[p]
        for g in range(2):
            psT = psT_p.tile([D, 512], FP16, tag="ps_t",
                             name=f"psT_{p}_{qh}_{hi}_{g}")
            for k in range(4):
                qt = 4 * g + k
                psO = psO_p.tile([128, 65], F32, tag="ps_o",
                                 name=f"psO_{p}_{qh}_{hi}_{qt}")
                # rotate accumulation order per group: later groups start at
                # later l so no group serializes on the newest exps (PSUM
                # accumulation is order-independent)
                ls = [(2 * qt + i) % LCH for i in range(LCH)]
                for j, l in enumerate(ls):
                    nc.tensor.matmul(
                        psO[:, :],
                        lhsT=wts[hi][l][:, 128 * qt:128 * (qt + 1)],
                        rhs=st["vhat"][hi][:, 65 * l:65 * (l + 1)],
                        start=(j == 0), stop=(j == LCH - 1))
                rz = rz_p.tile([128, 1], F32, tag="rz",
                               name=f"rz_{p}_{qh}_{hi}_{qt}")
                nc.vector.reciprocal(rz[:, :], psO[:, 64:65])
                ob = obar_p.tile([128, D], FP16, tag="obar",
                                 name=f"ob_{p}_{qh}_{hi}_{qt}")
                nc.vector.tensor_scalar_mul(ob[:, :], psO[:, 0:D], rz[:, 0:1])
                nc.tensor.transpose(psT[:, 128 * k:128 * (k + 1)],
                                    ob[:, :], ident[:, :])
                l_nxt = next(unit_iter, None)
                if l_nxt is not None:
                    emit_S_unit(nxt, l_nxt, nxt_wts)
            nc.vector.tensor_copy(
                st["aT"][hi][:, 1024 * qh + 512 * g:1024 * qh + 512 * (g + 1)],
                psT[:, :])

    def emit_Wo(p, hi, unit_iter=iter(()), nxt=None, nxt_wts=None):
        """output projection for head 2p+hi (needs aT[hi] complete).
        out row 128h+b uses head h features A_h[16b+t, d] -> Wo.T[64t+d]:
        out[128h.., e] = sum_t A_h.T[:, t::16].T @ WoT[64t:64t+64, :]"""
        h = 2 * p + hi
        aTr = pair_state[p]["aT"][hi][:, :].rearrange("d (b t) -> d t b", t=16)
        for e in range(2):
            es = slice(512 * e, 512 * (e + 1))
            psW = psO_p.tile([128, 512], F32, tag="ps_o", name=f"psW_{h}_{e}")
            for t in range(16):
                nc.tensor.matmul(psW[:, :], lhsT=aTr[:, t, :],
                                 rhs=woT[t][:, es],
                                 start=(t == 0), stop=(t == 15))
            ov = oev_p.tile([128, 512], F32, tag="oev", name=f"ov_{h}_{e}")
            nc.vector.tensor_copy(ov[:, :], psW[:, :])
            nc.sync.dma_start(out_d[128 * h:128 * (h + 1), es], ov[:, :])
            l_nxt = next(unit_iter, None)
            if l_nxt is not None:
                emit_S_unit(nxt, l_nxt, nxt_wts)

    # Software pipeline over 8 stages (pair, q-half): the next stage's
    # S/exp/mask work is emitted between the current stage's two per-head
    # O-phases so the in-order PE stream always has exp producers queued
    # while O-accumulation runs (keeps ScalarE, the bottleneck, saturated).
    stages = [(p, qh) for p in range(PAIRS) for qh in range(2)]
    # warm up ScalarE first: the one-time exp table-set load (~2.7us) runs
    # against a memset scratch tile while the first input DMAs are in flight
    warm = obar_p.tile([128, 1], FP16, tag="obar", name="act_warm")
    nc.gpsimd.memset(warm[:, :], 0.0)
    nc.scalar.activation(warm[:, :], warm[:, :], Exp)
    load_pair(0)
    cur = [[None] * LCH, [None] * LCH]
    for l in range(LCH):
        emit_S_unit(stages[0], l, cur)
    nc.sync.dma_start(ident[:, :], id_d[:, :])
    for t in range(16):
        nc.sync.dma_start(woT[t][:, :], woT_d[t, :, :])

    for idx, stage in enumerate(stages):
        p, qh = stage
        nxt = stages[idx + 1] if idx + 1 < len(stages) else None
        nxt_wts = [[None] * LCH, [None] * LCH] if nxt else None
        if nxt and nxt[1] == 0:
            load_pair(nxt[0])
        # PE load balancing: a qh==1 stage owns two Wo blocks (~13.6us PE)
        # while qh==0 owns none; defer Wo(p, 0) into stage (p+1, 0) so every
        # stage carries one Wo block and the exp-unit supply stays matched.
        # (p=3 has no following stage, so its Wo(3,0) stays in place.)
        unit_iter = iter(range(LCH)) if nxt else iter(())
        emit_O(stage, 0, cur, unit_iter, nxt, nxt_wts)
        if qh == 0 and p > 0:
            emit_Wo(p - 1, 0, unit_iter, nxt, nxt_wts)
        if qh == 1 and p == PAIRS - 1:
            emit_Wo(p, 0, unit_iter, nxt, nxt_wts)
        emit_O(stage, 1, cur, unit_iter, nxt, nxt_wts)
        if qh == 1:
            emit_Wo(p, 1, unit_iter, nxt, nxt_wts)
        cur = nxt_wts


_NC_CACHE = None


def get_nc():
    global _NC_CACHE
    if _NC_CACHE is None:
        _NC_CACHE = build_program()
    return _NC_CACHE


def make_in_maps(keys, values, queries, mask, Wk, Wv, Wq, Wo, bo):
    keys = np.asarray(keys, np.float32)
    values = np.asarray(values, np.float32)
    queries = np.asarray(queries, np.float32)
    mask = np.asarray(mask)
    Wk = np.asarray(Wk, np.float32)
    Wv = np.asarray(Wv, np.float32)
    Wq = np.asarray(Wq, np.float32)
    Wo = np.asarray(Wo, np.float32)

    ident = np.eye(128, dtype=np.float16)
    woT = np.ascontiguousarray(Wo.T.astype(np.float16)).reshape(16, D, EMBED)
    wq_s = (Wq / 32.0).astype(np.float32)           # fold 1/sqrt(EMBED) into q

    in_maps = []
    for n in range(N_BATCH):
        maskT = np.ascontiguousarray(mask[n, 0].T).astype(np.float16)
        for half in range(2):
            rows = slice(half * 1024, (half + 1) * 1024)
            # heads of this core as [8, 2048, 64] blocks
            qb = queries[n, rows, :].reshape(HPC, SEQ, D)
            kb = keys[n, rows, :].reshape(HPC, SEQ, D)
            vb = values[n, rows, :].reshape(HPC, SEQ, D)
            # host projections: qhatT/khatT as [8, 64(dout), 2048(l)]
            qhatT = np.einsum("od,hld->hol", wq_s, qb).astype(np.float16)
            khatT = np.einsum("od,hld->hol", Wk, kb).astype(np.float16)
            vhat = vb @ Wv.T                        # [8, 2048, 64] f32
            vext = np.empty((HPC, SEQ, 65), np.float16)
            vext[:, :, :D] = vhat.astype(np.float16)
            vext[:, :, D] = 1.0
            # device layout [8, 128, 16*65]: row p, block j -> l = 128*j + p
            vsh = np.ascontiguousarray(
                vext.reshape(HPC, 16, 128, 65).transpose(0, 2, 1, 3)
            ).reshape(HPC, 128, 16 * 65)
            in_maps.append({
                "qhatT": np.ascontiguousarray(qhatT),
                "khatT": np.ascontiguousarray(khatT),
                "vhat": vsh, "maskT": maskT,
                "woT": woT, "ident": ident,
            })
    return in_maps


def kernel(keys, values, queries, mask, Wk, Wv, Wq, Wo, bo):
    from concourse.bass_utils import run_bass_kernel_spmd

    nc = get_nc()
    in_maps = make_in_maps(keys, values, queries, mask, Wk, Wv, Wq, Wo, bo)
    res = run_bass_kernel_spmd(nc, in_maps, core_ids=list(range(N_CORES)))
    parts = [r["out"] for r in res.results]
    bo = np.asarray(bo, np.float32)
    out = np.empty((N_BATCH, SEQ, EMBED), np.float32)
    for n in range(N_BATCH):
        out[n, :1024] = parts[2 * n] + bo
        out[n, 1024:] = parts[2 * n + 1] + bo
    return out
